# revision 1
# baseline (speedup 1.0000x reference)
"""GQA attention kernel for Trainium2 (8 NeuronCores).

Problem: B=2, S=2048, D=2048, H=16 heads of DH=128, KV=4 kv heads, G=4
query heads per kv head.  Full (dense) attention, fp32 I/O.

Sharding: batch (2) x kv-head (4) = 8 cores, zero redundant FLOPs.
Each core computes, for its (batch b, kv head h):
    Q_g = x_b @ Wq[:, h,g]  (4 query heads), K = x_b @ Wk[:, h],
    V = x_b @ Wv[:, h], O_g = softmax(Q_g K^T / sqrt(DH)) V,
    y_partial = concat_g(O_g) @ Wo[h-rows, :]
Host sums the 4 kv-head partials per batch and adds bo.

On-chip strategy (all matmuls bf16 with fp32 PSUM accumulation):
 - host pre-transposes x (xT: [D, S]) and pre-casts weights to bf16
 - QT/KT computed head-transposed ([dh, s]) with W stationary, xT moving
 - S^T tiles ([k, q]) computed directly (KT-slice stationary, QT moving)
   so exp(S^T) lands in SBUF already transposed for the AV matmul:
   no P-transpose pass, no max-subtraction (scores are O(few), exp safe)
 - rowsum via ones-vector matmul accumulated alongside AV
 - softmax normalization folded into the PSUM->SBUF copy of O^T
   (multiply by DMA-broadcast 1/rowsum row)
 - out-proj: O^T stationary, Wo moving -> y natural, DMA'd straight
   from PSUM to DRAM.
"""

import sys

if "/opt/trn_rl_repo" not in sys.path:
    sys.path.insert(0, "/opt/trn_rl_repo")

import numpy as np
import ml_dtypes
from contextlib import ExitStack

B, S, D = 2, 2048, 2048
H, DH, GRP = 16, 128, 4
KV = H // GRP            # 4 kv heads
EH = GRP * DH            # 512 = query-head columns per kv head
SCALE = float(1.0 / np.sqrt(np.float32(DH)))
P = 128                  # partitions
NB = 512                 # matmul moving-dim block (one PSUM bank fp32)


def _emit(ctx, tc, aps, s=S, d=D, debug_taps=None):
    """Emit the per-core program. s, d parameterized for small-shape sim tests."""
    import concourse.bass as bass
    from concourse import mybir

    nc = tc.nc
    bf16 = mybir.dt.bfloat16
    f32 = mybir.dt.float32
    Exp = mybir.ActivationFunctionType.Exp
    Identity = mybir.ActivationFunctionType.Identity

    xt, wq, wk, wv, wo, bq, bk, bv, y = (
        aps["xt"], aps["wq"], aps["wk"], aps["wv"], aps["wo"],
        aps["bq"], aps["bk"], aps["bv"], aps["y"],
    )
    nt = s // P           # number of 128-tiles along s
    nd = d // P           # number of 128-tiles along d (contraction)
    nsb = s // NB         # number of 512-blocks along s
    ndb = d // NB         # number of 512-blocks along d (out columns)

    persist = ctx.enter_context(tc.tile_pool(name="persist", bufs=1))
    psum = ctx.enter_context(tc.tile_pool(name="psum", bufs=2, space="PSUM"))
    ptpool = ctx.enter_context(tc.tile_pool(name="ptp", bufs=2))
    rpool = ctx.enter_context(tc.tile_pool(name="rp", bufs=2))
    projpool = tc.tile_pool(name="projp", bufs=1)
    projp = projpool.__enter__()

    xt_sb = projp.tile([P, nd, s], bf16)
    wq_sb = projp.tile([P, nd, EH], bf16)
    wk_sb = projp.tile([P, nd, DH], bf16)
    wv_sb = projp.tile([P, nd, DH], bf16)
    wo_sb = persist.tile([P, GRP, d], bf16)
    qt_sb = persist.tile([P, GRP, s], bf16)
    kt_sb = persist.tile([P, s], bf16)
    v_sb = persist.tile([P, nt, DH], bf16)
    ot_sb = persist.tile([P, GRP, s], bf16)
    bq_sb = persist.tile([P, GRP], f32)
    bk_sb = persist.tile([P, 1], f32)
    bvb_sb = persist.tile([P, DH], f32)
    ones_sb = persist.tile([P, 1], bf16)

    nc.vector.memset(ones_sb, 1.0)

    # ---- loads ----
    xt_r = xt.rearrange("(t p) s -> p t s", p=P)
    wq_r = wq.rearrange("(t p) e -> p t e", p=P)
    wk_r = wk.rearrange("(t p) e -> p t e", p=P)
    wv_r = wv.rearrange("(t p) e -> p t e", p=P)
    wo_r = wo.rearrange("(g p) d -> p g d", p=P)
    for t in range(nd):
        nc.sync.dma_start(out=xt_sb[:, t, :], in_=xt_r[:, t, :])
        nc.sync.dma_start(out=wq_sb[:, t, :], in_=wq_r[:, t, :])
        nc.sync.dma_start(out=wk_sb[:, t, :], in_=wk_r[:, t, :])
        nc.sync.dma_start(out=wv_sb[:, t, :], in_=wv_r[:, t, :])
    for g in range(GRP):
        nc.sync.dma_start(out=wo_sb[:, g, :], in_=wo_r[:, g, :])
    nc.sync.dma_start(out=bq_sb, in_=bq.rearrange("(g p) -> p g", p=P))
    nc.sync.dma_start(out=bk_sb, in_=bk.rearrange("(p o) -> p o", o=1))
    # bv broadcast across partitions (varies along free dim of V)
    bv_bcast = bass.AP(tensor=bv.tensor, offset=bv.offset,
                       ap=[[0, P]] + list(bv.ap))
    nc.sync.dma_start(out=bvb_sb, in_=bv_bcast)

    # ---- projections ----
    # QT_g [dh, s] = (Wq_g)^T x^T, + bq*scale, scaled by 1/sqrt(DH)
    for g in range(GRP):
        for sb in range(nsb):
            ps = psum.tile([P, NB], f32, tag="mm")
            for t in range(nd):
                nc.tensor.matmul(
                    ps,
                    lhsT=wq_sb[:, t, g * DH:(g + 1) * DH],
                    rhs=xt_sb[:, t, sb * NB:(sb + 1) * NB],
                    start=(t == 0), stop=(t == nd - 1),
                )
            nc.scalar.activation(
                out=qt_sb[:, g, sb * NB:(sb + 1) * NB], in_=ps,
                func=Identity, bias=bq_sb[:, g:g + 1], scale=SCALE,
            )
    # KT [dh, s]
    for sb in range(nsb):
        ps = psum.tile([P, NB], f32, tag="mm")
        for t in range(nd):
            nc.tensor.matmul(
                ps, lhsT=wk_sb[:, t, :], rhs=xt_sb[:, t, sb * NB:(sb + 1) * NB],
                start=(t == 0), stop=(t == nd - 1),
            )
        nc.scalar.activation(
            out=kt_sb[:, sb * NB:(sb + 1) * NB], in_=ps,
            func=Identity, bias=bk_sb[:, 0:1], scale=1.0,
        )
    # V natural [k, dh] (xT stationary)
    for ki in range(nt):
        ps = psum.tile([P, NB], f32, tag="mm")
        for t in range(nd):
            nc.tensor.matmul(
                ps[:, 0:DH], lhsT=xt_sb[:, t, ki * P:(ki + 1) * P],
                rhs=wv_sb[:, t, :],
                start=(t == 0), stop=(t == nd - 1),
            )
        nc.vector.tensor_add(v_sb[:, ki, :], ps[:, 0:DH], bvb_sb)

    projpool.__exit__(None, None, None)

    # ---- attention ----
    # Software-pipelined: block n's S^T/exp interleave with block n-1's
    # AV + rowsum matmuls so PE never stalls waiting for ScalarE's exp.
    blocks = [(g, qb) for g in range(GRP) for qb in range(nsb)]

    def finish_block(prev):
        pg, pqb, ppt, pps_o, pps_r = prev
        pqsl = slice(pqb * NB, (pqb + 1) * NB)
        rrow = rpool.tile([1, NB], f32, tag="rrow")
        nc.vector.reciprocal(rrow, pps_r)
        rb = rpool.tile([P, NB], f32, tag="rb")
        nc.gpsimd.partition_broadcast(rb, rrow[0:1, :])
        nc.vector.tensor_mul(ot_sb[:, pg, pqsl], pps_o, rb)

    prev = None
    for g, qb in blocks:
        qsl = slice(qb * NB, (qb + 1) * NB)
        pt = ptpool.tile([P, nt, NB], bf16, tag="pt")
        ps_o = psum.tile([P, NB], f32, tag="o")
        ps_r = psum.tile([1, NB], f32, tag="r")
        for ki in range(nt):
            ps_s = psum.tile([P, NB], f32, tag="s")
            nc.tensor.matmul(
                ps_s, lhsT=kt_sb[:, ki * P:(ki + 1) * P],
                rhs=qt_sb[:, g, qsl], start=True, stop=True,
            )
            nc.scalar.activation(out=pt[:, ki, :], in_=ps_s, func=Exp)
            if prev is not None:
                _, _, ppt, pps_o, pps_r = prev
                nc.tensor.matmul(
                    pps_o, lhsT=v_sb[:, ki, :], rhs=ppt[:, ki, :],
                    start=(ki == 0), stop=(ki == nt - 1),
                )
                nc.tensor.matmul(
                    pps_r, lhsT=ones_sb[:, 0:1], rhs=ppt[:, ki, :],
                    start=(ki == 0), stop=(ki == nt - 1),
                )
        if prev is not None:
            finish_block(prev)
        prev = (g, qb, pt, ps_o, ps_r)
    # drain last block
    g, qb, pt, ps_o, ps_r = prev
    for ki in range(nt):
        nc.tensor.matmul(
            ps_o, lhsT=v_sb[:, ki, :], rhs=pt[:, ki, :],
            start=(ki == 0), stop=(ki == nt - 1),
        )
        nc.tensor.matmul(
            ps_r, lhsT=ones_sb[:, 0:1], rhs=pt[:, ki, :],
            start=(ki == 0), stop=(ki == nt - 1),
        )
    finish_block(prev)

    if debug_taps is not None:
        for name, t in [("qt", qt_sb), ("kt", kt_sb), ("v", v_sb),
                        ("ot", ot_sb), ("pt_last", None)]:
            if name in debug_taps and t is not None:
                nc.sync.dma_start(out=debug_taps[name], in_=t[:])

    # ---- out projection ----
    ypool = ctx.enter_context(tc.tile_pool(name="yp", bufs=2))
    for st in range(nt):
        for db in range(ndb):
            ps_y = psum.tile([P, NB], f32, tag="mm")
            for g in range(GRP):
                nc.tensor.matmul(
                    ps_y, lhsT=ot_sb[:, g, st * P:(st + 1) * P],
                    rhs=wo_sb[:, g, db * NB:(db + 1) * NB],
                    start=(g == 0), stop=(g == GRP - 1),
                )
            y_sb = ypool.tile([P, NB], f32, tag="y")
            if (st * ndb + db) % 2 == 0:
                nc.scalar.copy(y_sb, ps_y)
            else:
                nc.vector.tensor_copy(y_sb, ps_y)
            nc.sync.dma_start(
                out=y[st * P:(st + 1) * P, db * NB:(db + 1) * NB], in_=y_sb)


def build_program(s=S, d=D, debug=False):
    import concourse.tile as tile
    from concourse import bacc, mybir

    nc = bacc.Bacc("TRN2", target_bir_lowering=False, debug=False)
    bf16 = mybir.dt.bfloat16
    f32 = mybir.dt.float32
    aps = {
        "xt": nc.dram_tensor("xt", [d, s], bf16, kind="ExternalInput").ap(),
        "wq": nc.dram_tensor("wq", [d, EH], bf16, kind="ExternalInput").ap(),
        "wk": nc.dram_tensor("wk", [d, DH], bf16, kind="ExternalInput").ap(),
        "wv": nc.dram_tensor("wv", [d, DH], bf16, kind="ExternalInput").ap(),
        "wo": nc.dram_tensor("wo", [EH, d], bf16, kind="ExternalInput").ap(),
        "bq": nc.dram_tensor("bq", [EH], f32, kind="ExternalInput").ap(),
        "bk": nc.dram_tensor("bk", [DH], f32, kind="ExternalInput").ap(),
        "bv": nc.dram_tensor("bv", [DH], f32, kind="ExternalInput").ap(),
        "y": nc.dram_tensor("y", [s, d], f32, kind="ExternalOutput").ap(),
    }
    debug_taps = None
    if debug:
        nt = s // P
        debug_taps = {
            "qt": nc.dram_tensor("dbg_qt", [P, GRP, s], bf16, kind="ExternalOutput").ap(),
            "kt": nc.dram_tensor("dbg_kt", [P, s], bf16, kind="ExternalOutput").ap(),
            "v": nc.dram_tensor("dbg_v", [P, nt, DH], bf16, kind="ExternalOutput").ap(),
            "ot": nc.dram_tensor("dbg_ot", [P, GRP, s], bf16, kind="ExternalOutput").ap(),
        }
    with tile.TileContext(nc) as tc:
        with ExitStack() as ctx:
            _emit(ctx, tc, aps, s=s, d=d, debug_taps=debug_taps)
    nc.compile()
    return nc


def make_in_maps(x, Wq, bq, Wk, bk, Wv, bv, Wo, bo):
    bf = ml_dtypes.bfloat16
    in_maps = []
    for b in range(B):
        xt_b = x[b].T.astype(bf)  # [D, S] contiguous
        for h in range(KV):
            in_maps.append({
                "xt": xt_b,
                "wq": Wq[:, h * EH:(h + 1) * EH].astype(bf),
                "wk": Wk[:, h * DH:(h + 1) * DH].astype(bf),
                "wv": Wv[:, h * DH:(h + 1) * DH].astype(bf),
                "wo": np.ascontiguousarray(Wo[h * EH:(h + 1) * EH, :]).astype(bf),
                "bq": (bq[h * EH:(h + 1) * EH] * SCALE).astype(np.float32),
                "bk": np.ascontiguousarray(bk[h * DH:(h + 1) * DH]).astype(np.float32),
                "bv": np.ascontiguousarray(bv[h * DH:(h + 1) * DH]).astype(np.float32),
            })
    return in_maps


_PROG = None


def _get_program():
    global _PROG
    if _PROG is None:
        _PROG = build_program()
    return _PROG


def run_cores(in_maps, trace=False, **kw):
    from concourse.bass_utils import run_bass_kernel_spmd
    nc = _get_program()
    return run_bass_kernel_spmd(nc, in_maps, list(range(8)), trace=trace, **kw)


def kernel(**inputs):
    x = np.asarray(inputs["x"], dtype=np.float32)
    Wq = np.asarray(inputs["Wq"], dtype=np.float32)
    bq = np.asarray(inputs["bq"], dtype=np.float32)
    Wk = np.asarray(inputs["Wk"], dtype=np.float32)
    bk = np.asarray(inputs["bk"], dtype=np.float32)
    Wv = np.asarray(inputs["Wv"], dtype=np.float32)
    bv = np.asarray(inputs["bv"], dtype=np.float32)
    Wo = np.asarray(inputs["Wo"], dtype=np.float32)
    bo = np.asarray(inputs["bo"], dtype=np.float32)

    in_maps = make_in_maps(x, Wq, bq, Wk, bk, Wv, bv, Wo, bo)
    res = run_cores(in_maps)
    out = np.empty((B, S, D), dtype=np.float32)
    for b in range(B):
        acc = res.results[b * KV]["y"].astype(np.float32)
        for h in range(1, KV):
            acc = acc + res.results[b * KV + h]["y"]
        out[b] = acc + bo[None, :]
    return out



# revision 7
# speedup vs baseline: 1.4190x; 1.4190x over previous
"""GQA attention kernel for Trainium2 (8 NeuronCores).

Problem: B=2, S=2048, D=2048, H=16 heads of DH=128, KV=4 kv heads, G=4
query heads per kv head.  Full (dense) attention, fp32 I/O.

Sharding: batch (2) x kv-head (4) = 8 cores, zero redundant FLOPs.
Each core computes, for its (batch b, kv head h):
    Q_g = x_b @ Wq[:, h,g]  (4 query heads), K = x_b @ Wk[:, h],
    V = x_b @ Wv[:, h], O_g = softmax(Q_g K^T / sqrt(DH)) V,
    y_partial = concat_g(O_g) @ Wo[h-rows, :]
Host sums the 4 kv-head partials per batch and adds bo.

On-chip strategy (matmuls bf16 with fp32 PSUM accumulation; fp8 was
tried and measured: any e4m3 leg in the Q/K/P/V path alone costs
2.4e-2..4.8e-2 relative error because softmax output is a weighted
average -- per-element quantization error passes straight through):
 - host pre-transposes x (xT: [D, S]) and pre-casts weights to bf16
 - whole-tensor input DMAs (the cost model serializes ~600ns of HWDGE
   setup per DMA; 64 small loads would cost ~40us of setup)
 - QT/KT computed head-transposed ([dh, s]) with W stationary, xT
   moving; bias and 1/sqrt(DH) folded into the ACT PSUM->SBUF copy
 - S^T tiles ([k, q]) computed directly (KT-slice stationary) so
   exp(S^T) lands in SBUF already transposed for the AV matmul; two
   k-tiles share a 2-bank PSUM group so exp runs one ACT call per pair
 - rowsum runs OFF the PE: DVE accumulates exp tiles in fp16
   (cascade error ~5e-4), GpSimd partition_all_reduce finishes the
   cross-partition sum (result pre-broadcast on all 128 partitions).
   This saves the 54.6us ones-matmul pass the PE used to do.
 - AV matmuls software-pipelined against the next block's scores/exp;
   softmax normalization folded into the PSUM->SBUF copy of O^T
 - blocks run q-major so the out-projection of q-block n overlaps the
   attention of q-block n+1; y written bf16, one row-block DMA per
   128-row stripe; PSUM->SBUF y copies alternate ACT/DVE.
"""

import sys

if "/opt/trn_rl_repo" not in sys.path:
    sys.path.insert(0, "/opt/trn_rl_repo")

import numpy as np
import ml_dtypes
from contextlib import ExitStack

B, S, D = 2, 2048, 2048
H, DH, GRP = 16, 128, 4
KV = H // GRP            # 4 kv heads
EH = GRP * DH            # 512 = query-head columns per kv head
SCALE = float(1.0 / np.sqrt(np.float32(DH)))
P = 128                  # partitions
NB = 512                 # matmul moving-dim block (one PSUM bank fp32)


def _emit(ctx, tc, aps, s=S, d=D):
    """Emit the per-core program. s, d parameterized for small-shape sim tests."""
    import concourse.bass as bass
    from concourse import mybir
    from concourse import bass_isa

    nc = tc.nc
    bf16 = mybir.dt.bfloat16
    fp16 = mybir.dt.float16
    f32 = mybir.dt.float32
    Exp = mybir.ActivationFunctionType.Exp
    Identity = mybir.ActivationFunctionType.Identity

    xt, wq, wk, wv, wo, bq, bk, bv, y = (
        aps["xt"], aps["wq"], aps["wk"], aps["wv"], aps["wo"],
        aps["bq"], aps["bk"], aps["bv"], aps["y"],
    )
    nt = s // P           # number of 128-tiles along s
    nd = d // P           # number of 128-tiles along d (contraction)
    nkp = nt // 2         # k-tile pairs (PSUM 2-bank score groups)
    nsb = s // NB         # number of 512-blocks along s
    ndb = d // NB         # number of 512-blocks along d (out columns)

    persist = ctx.enter_context(tc.tile_pool(name="persist", bufs=1))
    psum = ctx.enter_context(tc.tile_pool(name="psum", bufs=1, space="PSUM"))
    ptpool = ctx.enter_context(tc.tile_pool(name="ptp", bufs=2))
    rpool = ctx.enter_context(tc.tile_pool(name="rp", bufs=2))
    ypool = ctx.enter_context(tc.tile_pool(name="yp", bufs=2))
    projpool = tc.tile_pool(name="projp", bufs=1)
    projp = projpool.__enter__()

    xt_sb = projp.tile([P, nd, s], bf16)
    wq_sb = projp.tile([P, nd, EH], bf16)
    wk_sb = projp.tile([P, nd, DH], bf16)
    wv_sb = projp.tile([P, nd, DH], bf16)
    wo_sb = persist.tile([P, GRP, d], bf16)
    qt_sb = persist.tile([P, GRP, s], bf16)
    kt_sb = persist.tile([P, s], bf16)
    v_sb = persist.tile([P, nt, DH], bf16)
    ot_sb = persist.tile([P, GRP, s], bf16)
    bq_sb = persist.tile([P, GRP], f32)
    bk_sb = persist.tile([P, 1], f32)
    bvb_sb = persist.tile([P, DH], f32)

    # ---- loads (one DMA per tensor; HWDGE setup is ~600ns serialized) ----
    nc.sync.dma_start(out=xt_sb, in_=xt.rearrange("(t p) s -> p t s", p=P))
    nc.sync.dma_start(out=wk_sb, in_=wk.rearrange("(t p) e -> p t e", p=P))
    nc.sync.dma_start(out=wq_sb, in_=wq.rearrange("(t p) e -> p t e", p=P))
    nc.sync.dma_start(out=wv_sb, in_=wv.rearrange("(t p) e -> p t e", p=P))
    nc.sync.dma_start(out=wo_sb, in_=wo.rearrange("(g p) d -> p g d", p=P))
    nc.sync.dma_start(out=bq_sb, in_=bq.rearrange("(g p) -> p g", p=P))
    nc.sync.dma_start(out=bk_sb, in_=bk.rearrange("(p o) -> p o", o=1))
    # bv broadcast across partitions (varies along free dim of V)
    bv_bcast = bass.AP(tensor=bv.tensor, offset=bv.offset,
                       ap=[[0, P]] + list(bv.ap))
    nc.sync.dma_start(out=bvb_sb, in_=bv_bcast)

    # ---- projections ----
    # KT [dh, s]
    for sb in range(nsb):
        ps = psum.tile([P, NB], f32, tag="y", bufs=2)
        for t in range(nd):
            nc.tensor.matmul(
                ps, lhsT=wk_sb[:, t, :], rhs=xt_sb[:, t, sb * NB:(sb + 1) * NB],
                start=(t == 0), stop=(t == nd - 1),
            )
        nc.scalar.activation(
            out=kt_sb[:, sb * NB:(sb + 1) * NB], in_=ps,
            func=Identity, bias=bk_sb[:, 0:1], scale=1.0,
        )

    # QT_g [dh, s] = (Wq_g)^T x^T scaled by 1/sqrt(DH), + bq*scale
    def emit_qt(g):
        for sb in range(nsb):
            ps = psum.tile([P, NB], f32, tag="y", bufs=2)
            for t in range(nd):
                nc.tensor.matmul(
                    ps, lhsT=wq_sb[:, t, g * DH:(g + 1) * DH],
                    rhs=xt_sb[:, t, sb * NB:(sb + 1) * NB],
                    start=(t == 0), stop=(t == nd - 1),
                )
            nc.scalar.activation(
                out=qt_sb[:, g, sb * NB:(sb + 1) * NB], in_=ps,
                func=Identity, bias=bq_sb[:, g:g + 1], scale=SCALE,
            )

    emit_qt(0)
    # V natural [k, dh] (xT stationary)
    for ki in range(nt):
        ps = psum.tile([P, NB], f32, tag="y", bufs=2)
        for t in range(nd):
            nc.tensor.matmul(
                ps[:, 0:DH], lhsT=xt_sb[:, t, ki * P:(ki + 1) * P],
                rhs=wv_sb[:, t, :],
                start=(t == 0), stop=(t == nd - 1),
            )
        nc.vector.tensor_add(v_sb[:, ki, :], ps[:, 0:DH], bvb_sb)
    for g in range(1, GRP):
        emit_qt(g)

    projpool.__exit__(None, None, None)

    # ---- attention (q-major) + interleaved out-projection ----
    blocks = [(qb, g) for qb in range(nsb) for g in range(GRP)]

    def finish_block(prev):
        pqb, pg, ppt, pps_o, pracc = prev
        pqsl = slice(pqb * NB, (pqb + 1) * NB)
        red = rpool.tile([P, NB], f32, tag="red")
        nc.gpsimd.partition_all_reduce(red, pracc, channels=P,
                                       reduce_op=bass_isa.ReduceOp.add)
        rb = rpool.tile([P, NB], f32, tag="rb")
        nc.vector.reciprocal(rb, red)
        nc.vector.tensor_mul(ot_sb[:, pg, pqsl], pps_o, rb)

    ncopy = [0]

    def emit_outproj_st(st):
        y_sb = ypool.tile([P, d], bf16, tag="ysb")
        for db in range(ndb):
            ps_y = psum.tile([P, NB], f32, tag="y", bufs=2)
            for g2 in range(GRP):
                nc.tensor.matmul(
                    ps_y, lhsT=ot_sb[:, g2, st * P:(st + 1) * P],
                    rhs=wo_sb[:, g2, db * NB:(db + 1) * NB],
                    start=(g2 == 0), stop=(g2 == GRP - 1),
                )
            ysl = y_sb[:, db * NB:(db + 1) * NB]
            if ncopy[0] % 2 == 0:
                nc.scalar.copy(ysl, ps_y)
            else:
                nc.vector.tensor_copy(ysl, ps_y)
            ncopy[0] += 1
        nc.sync.dma_start(out=y[st * P:(st + 1) * P, :], in_=y_sb)

    sts_pending = []
    prev = None
    for qb, g in blocks:
        qsl = slice(qb * NB, (qb + 1) * NB)
        pt = ptpool.tile([P, nt, NB], bf16, tag="pt")
        ps_o = psum.tile([P, NB], f32, tag="o", bufs=2)
        racc = rpool.tile([P, NB], fp16, tag="racc")
        for kj in range(nkp):
            ps_s = psum.tile([P, 2, NB], f32, tag="s", bufs=2)
            for i in range(2):
                ki = 2 * kj + i
                nc.tensor.matmul(
                    ps_s[:, i, :], lhsT=kt_sb[:, ki * P:(ki + 1) * P],
                    rhs=qt_sb[:, g, qsl], start=True, stop=True,
                )
            nc.scalar.activation(
                out=pt[:, 2 * kj:2 * kj + 2, :], in_=ps_s, func=Exp)
            # rowsum of this block's fresh exp pair on DVE (fp16 cascade)
            if kj == 0:
                nc.vector.tensor_add(racc, pt[:, 0, :], pt[:, 1, :])
            else:
                nc.vector.tensor_add(racc, racc, pt[:, 2 * kj, :])
                nc.vector.tensor_add(racc, racc, pt[:, 2 * kj + 1, :])
            if prev is not None:
                ppt = prev[2]
                pps_o = prev[3]
                for i in range(2):
                    ki = 2 * kj + i
                    nc.tensor.matmul(
                        pps_o, lhsT=v_sb[:, ki, :], rhs=ppt[:, ki, :],
                        start=(ki == 0), stop=(ki == nt - 1),
                    )
        if prev is not None:
            finish_block(prev)
            if prev[1] == GRP - 1:
                pqb = prev[0]
                sts_pending.extend(range(pqb * (NB // P), (pqb + 1) * (NB // P)))
        if sts_pending:
            emit_outproj_st(sts_pending.pop(0))
        prev = (qb, g, pt, ps_o, racc)
    # drain last block
    qb, g, pt, ps_o, racc = prev
    for ki in range(nt):
        nc.tensor.matmul(
            ps_o, lhsT=v_sb[:, ki, :], rhs=pt[:, ki, :],
            start=(ki == 0), stop=(ki == nt - 1),
        )
    finish_block(prev)
    sts_pending.extend(range(qb * (NB // P), (qb + 1) * (NB // P)))
    for st in sts_pending:
        emit_outproj_st(st)


def build_program(s=S, d=D):
    import concourse.tile as tile
    from concourse import bacc, mybir

    nc = bacc.Bacc("TRN2", target_bir_lowering=False, debug=False)
    bf16 = mybir.dt.bfloat16
    f32 = mybir.dt.float32
    aps = {
        "xt": nc.dram_tensor("xt", [d, s], bf16, kind="ExternalInput").ap(),
        "wq": nc.dram_tensor("wq", [d, EH], bf16, kind="ExternalInput").ap(),
        "wk": nc.dram_tensor("wk", [d, DH], bf16, kind="ExternalInput").ap(),
        "wv": nc.dram_tensor("wv", [d, DH], bf16, kind="ExternalInput").ap(),
        "wo": nc.dram_tensor("wo", [EH, d], bf16, kind="ExternalInput").ap(),
        "bq": nc.dram_tensor("bq", [EH], f32, kind="ExternalInput").ap(),
        "bk": nc.dram_tensor("bk", [DH], f32, kind="ExternalInput").ap(),
        "bv": nc.dram_tensor("bv", [DH], f32, kind="ExternalInput").ap(),
        "y": nc.dram_tensor("y", [s, d], bf16, kind="ExternalOutput").ap(),
    }
    with tile.TileContext(nc) as tc:
        with ExitStack() as ctx:
            _emit(ctx, tc, aps, s=s, d=d)
    nc.compile()
    return nc


def make_in_maps(x, Wq, bq, Wk, bk, Wv, bv, Wo, bo):
    bf = ml_dtypes.bfloat16
    in_maps = []
    for b in range(B):
        xt_b = x[b].T.astype(bf)  # [D, S] contiguous
        for h in range(KV):
            in_maps.append({
                "xt": xt_b,
                "wq": Wq[:, h * EH:(h + 1) * EH].astype(bf),
                "wk": Wk[:, h * DH:(h + 1) * DH].astype(bf),
                "wv": Wv[:, h * DH:(h + 1) * DH].astype(bf),
                "wo": np.ascontiguousarray(Wo[h * EH:(h + 1) * EH, :]).astype(bf),
                "bq": (bq[h * EH:(h + 1) * EH] * SCALE).astype(np.float32),
                "bk": np.ascontiguousarray(bk[h * DH:(h + 1) * DH]).astype(np.float32),
                "bv": np.ascontiguousarray(bv[h * DH:(h + 1) * DH]).astype(np.float32),
            })
    return in_maps


_PROG = None


def _get_program():
    global _PROG
    if _PROG is None:
        _PROG = build_program()
    return _PROG


def run_cores(in_maps, trace=False, **kw):
    from concourse.bass_utils import run_bass_kernel_spmd
    nc = _get_program()
    return run_bass_kernel_spmd(nc, in_maps, list(range(8)), trace=trace, **kw)


def kernel(**inputs):
    x = np.asarray(inputs["x"], dtype=np.float32)
    Wq = np.asarray(inputs["Wq"], dtype=np.float32)
    bq = np.asarray(inputs["bq"], dtype=np.float32)
    Wk = np.asarray(inputs["Wk"], dtype=np.float32)
    bk = np.asarray(inputs["bk"], dtype=np.float32)
    Wv = np.asarray(inputs["Wv"], dtype=np.float32)
    bv = np.asarray(inputs["bv"], dtype=np.float32)
    Wo = np.asarray(inputs["Wo"], dtype=np.float32)
    bo = np.asarray(inputs["bo"], dtype=np.float32)

    in_maps = make_in_maps(x, Wq, bq, Wk, bk, Wv, bv, Wo, bo)
    res = run_cores(in_maps)
    out = np.empty((B, S, D), dtype=np.float32)
    for b in range(B):
        acc = res.results[b * KV]["y"].astype(np.float32)
        for h in range(1, KV):
            acc = acc + res.results[b * KV + h]["y"].astype(np.float32)
        out[b] = acc + bo[None, :]
    return out


# revision 10
# speedup vs baseline: 1.4949x; 1.0535x over previous
"""GQA attention kernel for Trainium2 (8 NeuronCores).

Problem: B=2, S=2048, D=2048, H=16 heads of DH=128, KV=4 kv heads, G=4
query heads per kv head.  Full (dense) attention, fp32 I/O.

Sharding: batch (2) x kv-head (4) = 8 cores, zero redundant FLOPs.
Each core computes, for its (batch b, kv head h):
    Q_g = x_b @ Wq[:, h,g]  (4 query heads), K = x_b @ Wk[:, h],
    V = x_b @ Wv[:, h], O_g = softmax(Q_g K^T / sqrt(DH)) V,
    y_partial = concat_g(O_g) @ Wo[h-rows, :]
Host sums the 4 kv-head partials per batch and adds bo.

On-chip strategy:
 - QKV projections run as two-level fp8 DoubleRow matmuls: host splits
   8*x and 256*W each into fp8-e4m3 hi + lo parts; hi*hi + hi*lo +
   lo*hi accumulate in PSUM at a common scale (/2048 folded into the
   PSUM->SBUF copy), lo*lo is dropped (~0.2% coherent).  24 DR passes
   replace 32 bf16 passes per group (0.75x PE), with error ~2x BETTER
   than bf16 inputs.  Single-level fp8 anywhere in the Q/K/P/V path
   was measured at 2.4e-2..4.8e-2 rel error (softmax output is a
   weighted average; per-element quantization passes straight
   through), hence two-level.
 - attention matmuls bf16: S^T tiles ([k, q]) computed directly
   (KT-slice stationary) so exp(S^T) lands in SBUF already transposed
   for the AV matmul; two k-tiles share a 2-bank PSUM group so exp
   runs one ACT call per pair.
 - rowsum runs OFF the PE: DVE accumulates exp tiles in fp16 (cascade
   error ~5e-4), GpSimd partition_all_reduce finishes the
   cross-partition sum (result pre-broadcast on all 128 partitions).
   Saves the 54.6us ones-matmul pass.
 - AV matmuls software-pipelined against the next block's scores/exp;
   softmax normalization folded into the PSUM->SBUF copy of O^T.
 - blocks run q-major; the out-projection (bf16) is fed two matmuls
   per score-pair from a work queue, filling the ~180ns/pair PE
   bubbles left by exp's 1038ns/pair ACT latency; y written bf16,
   one row-block DMA per 128-row stripe, copies alternate ACT/DVE.
 - whole-tensor input DMAs (HWDGE setup is ~600ns serialized per DMA).
"""

import sys

if "/opt/trn_rl_repo" not in sys.path:
    sys.path.insert(0, "/opt/trn_rl_repo")

import numpy as np
import ml_dtypes
from contextlib import ExitStack

B, S, D = 2, 2048, 2048
H, DH, GRP = 16, 128, 4
KV = H // GRP            # 4 kv heads
EH = GRP * DH            # 512 = query-head columns per kv head
SCALE = float(1.0 / np.sqrt(np.float32(DH)))
P = 128                  # partitions
NB = 512                 # matmul moving-dim block (one PSUM bank fp32)
XSC = 8.0                # fp8 two-level pre-scale for x
WSC = 256.0              # fp8 two-level pre-scale for Wq/Wk/Wv
PSC = XSC * WSC          # combined PSUM scale to compensate


def _emit(ctx, tc, aps, s=S, d=D):
    """Emit the per-core program. s, d parameterized for small-shape sim tests."""
    import concourse.bass as bass
    from concourse import mybir
    from concourse import bass_isa

    nc = tc.nc
    bf16 = mybir.dt.bfloat16
    fp16 = mybir.dt.float16
    fp8 = mybir.dt.float8e4
    f32 = mybir.dt.float32
    Exp = mybir.ActivationFunctionType.Exp
    Identity = mybir.ActivationFunctionType.Identity
    DR = mybir.MatmulPerfMode.DoubleRow
    Mult = mybir.AluOpType.mult
    Add = mybir.AluOpType.add

    xth, xtl, wqh, wql, wkh, wkl, wvh, wvl, wo, bq, bk, bv, y = (
        aps["xth"], aps["xtl"], aps["wqh"], aps["wql"], aps["wkh"],
        aps["wkl"], aps["wvh"], aps["wvl"], aps["wo"],
        aps["bq"], aps["bk"], aps["bv"], aps["y"],
    )
    nt = s // P           # number of 128-tiles along s
    nd = d // P           # number of 128-tiles along d (contraction)
    npair = nd // 2       # d-pairs for DoubleRow
    nkp = nt // 2         # k-tile pairs (PSUM 2-bank score groups)
    nsb = s // NB         # number of 512-blocks along s
    ndb = d // NB         # number of 512-blocks along d (out columns)

    persist = ctx.enter_context(tc.tile_pool(name="persist", bufs=1))
    psum = ctx.enter_context(tc.tile_pool(name="psum", bufs=1, space="PSUM"))
    ptpool = ctx.enter_context(tc.tile_pool(name="ptp", bufs=2))
    rpool = ctx.enter_context(tc.tile_pool(name="rp", bufs=2))
    ypool = ctx.enter_context(tc.tile_pool(name="yp", bufs=2))
    projpool = tc.tile_pool(name="projp", bufs=1)
    projp = projpool.__enter__()

    xth_sb = projp.tile([P, nd, s], fp8)
    xtl_sb = projp.tile([P, nd, s], fp8)
    wqh_sb = projp.tile([P, nd, EH], fp8)
    wql_sb = projp.tile([P, nd, EH], fp8)
    wkh_sb = projp.tile([P, nd, DH], fp8)
    wkl_sb = projp.tile([P, nd, DH], fp8)
    wvh_sb = projp.tile([P, nd, DH], fp8)
    wvl_sb = projp.tile([P, nd, DH], fp8)
    wo_sb = persist.tile([P, GRP, d], bf16)
    qt_sb = persist.tile([P, GRP, s], bf16)
    kt_sb = persist.tile([P, s], bf16)
    v_sb = persist.tile([P, nt, DH], bf16)
    ot_sb = persist.tile([P, GRP, s], bf16)
    bq_sb = persist.tile([P, GRP], f32)
    bk_sb = persist.tile([P, 1], f32)
    bvb_sb = persist.tile([P, DH], f32)

    # ---- loads (one DMA per tensor; HWDGE setup is ~600ns serialized) ----
    nc.sync.dma_start(out=xth_sb, in_=xth.rearrange("(t p) s -> p t s", p=P))
    nc.sync.dma_start(out=xtl_sb, in_=xtl.rearrange("(t p) s -> p t s", p=P))
    nc.sync.dma_start(out=wkh_sb, in_=wkh.rearrange("(t p) e -> p t e", p=P))
    nc.sync.dma_start(out=wkl_sb, in_=wkl.rearrange("(t p) e -> p t e", p=P))
    nc.sync.dma_start(out=wqh_sb, in_=wqh.rearrange("(t p) e -> p t e", p=P))
    nc.sync.dma_start(out=wql_sb, in_=wql.rearrange("(t p) e -> p t e", p=P))
    nc.sync.dma_start(out=wvh_sb, in_=wvh.rearrange("(t p) e -> p t e", p=P))
    nc.sync.dma_start(out=wvl_sb, in_=wvl.rearrange("(t p) e -> p t e", p=P))
    nc.sync.dma_start(out=wo_sb, in_=wo.rearrange("(g p) d -> p g d", p=P))
    nc.sync.dma_start(out=bq_sb, in_=bq.rearrange("(g p) -> p g", p=P))
    nc.sync.dma_start(out=bk_sb, in_=bk.rearrange("(p o) -> p o", o=1))
    # bv broadcast across partitions (varies along free dim of V)
    bv_bcast = bass.AP(tensor=bv.tensor, offset=bv.offset,
                       ap=[[0, P]] + list(bv.ap))
    nc.sync.dma_start(out=bvb_sb, in_=bv_bcast)

    # ---- projections: two-level fp8 DoubleRow over d-pairs ----
    def emit_proj_group(ps, wh_sb, wl_sb, wcols, xsl):
        """ps += (Wh+Wl)^T (xh+xl) over all d-pairs, lo*lo dropped."""
        terms = [(wh_sb, xth_sb), (wl_sb, xth_sb), (wh_sb, xtl_sb)]
        first = True
        for wsb, xsb in terms:
            for j in range(npair):
                nc.tensor.matmul(
                    ps, lhsT=wsb[:, 2 * j:2 * j + 2, wcols],
                    rhs=xsb[:, 2 * j:2 * j + 2, xsl],
                    start=first, stop=(wsb is wh_sb and xsb is xtl_sb
                                       and j == npair - 1),
                    perf_mode=DR,
                )
                first = False

    # KT [dh, s]
    for sb in range(nsb):
        ps = psum.tile([P, NB], f32, tag="y", bufs=2)
        emit_proj_group(ps, wkh_sb, wkl_sb, slice(0, DH),
                        slice(sb * NB, (sb + 1) * NB))
        nc.scalar.activation(
            out=kt_sb[:, sb * NB:(sb + 1) * NB], in_=ps,
            func=Identity, bias=bk_sb[:, 0:1], scale=1.0 / PSC,
        )

    # QT_g [dh, s] = (Wq_g)^T x^T scaled by 1/sqrt(DH), + bq*scale
    def emit_qt(g):
        for sb in range(nsb):
            ps = psum.tile([P, NB], f32, tag="y", bufs=2)
            emit_proj_group(ps, wqh_sb, wql_sb, slice(g * DH, (g + 1) * DH),
                            slice(sb * NB, (sb + 1) * NB))
            nc.scalar.activation(
                out=qt_sb[:, g, sb * NB:(sb + 1) * NB], in_=ps,
                func=Identity, bias=bq_sb[:, g:g + 1], scale=SCALE / PSC,
            )

    emit_qt(0)
    # V natural [k, dh] (xT stationary): (xh+xl)^T (Wvh+Wvl), lo*lo dropped
    for ki in range(nt):
        ps = psum.tile([P, NB], f32, tag="y", bufs=2)
        terms = [(xth_sb, wvh_sb), (xth_sb, wvl_sb), (xtl_sb, wvh_sb)]
        first = True
        for xsb, wsb in terms:
            for j in range(npair):
                nc.tensor.matmul(
                    ps[:, 0:DH], lhsT=xsb[:, 2 * j:2 * j + 2, ki * P:(ki + 1) * P],
                    rhs=wsb[:, 2 * j:2 * j + 2, :],
                    start=first, stop=(xsb is xtl_sb and j == npair - 1),
                    perf_mode=DR,
                )
                first = False
        nc.vector.scalar_tensor_tensor(
            v_sb[:, ki, :], ps[:, 0:DH], 1.0 / PSC, bvb_sb, Mult, Add)
    for g in range(1, GRP):
        emit_qt(g)

    projpool.__exit__(None, None, None)

    # ---- attention (q-major) + out-projection fed into PE bubbles ----
    blocks = [(qb, g) for qb in range(nsb) for g in range(GRP)]

    def finish_block(prev):
        pqb, pg, ppt, pps_o, pracc = prev
        pqsl = slice(pqb * NB, (pqb + 1) * NB)
        red = rpool.tile([P, NB], f32, tag="red")
        nc.gpsimd.partition_all_reduce(red, pracc, channels=P,
                                       reduce_op=bass_isa.ReduceOp.add)
        rb = rpool.tile([P, NB], f32, tag="rb")
        nc.vector.reciprocal(rb, red)
        nc.vector.tensor_mul(ot_sb[:, pg, pqsl], pps_o, rb)

    # out-projection work queue: one matmul per op_step() call
    sts_pending = []
    op_state = {"st": None, "db": 0, "g2": 0, "ysb": None, "psy": None, "n": 0}

    def op_step():
        stt = op_state
        if stt["st"] is None:
            if not sts_pending:
                return
            stt["st"] = sts_pending.pop(0)
            stt["db"] = 0
            stt["g2"] = 0
            stt["ysb"] = ypool.tile([P, d], bf16, tag="ysb", name="ysb")
        st, db, g2 = stt["st"], stt["db"], stt["g2"]
        if g2 == 0:
            stt["psy"] = psum.tile([P, NB], f32, tag="y", bufs=2, name="psy")
        nc.tensor.matmul(
            stt["psy"], lhsT=ot_sb[:, g2, st * P:(st + 1) * P],
            rhs=wo_sb[:, g2, db * NB:(db + 1) * NB],
            start=(g2 == 0), stop=(g2 == GRP - 1),
        )
        if g2 == GRP - 1:
            ysl = stt["ysb"][:, db * NB:(db + 1) * NB]
            if stt["n"] % 2 == 0:
                nc.scalar.copy(ysl, stt["psy"])
            else:
                nc.vector.tensor_copy(ysl, stt["psy"])
            stt["n"] += 1
            if db == ndb - 1:
                nc.sync.dma_start(out=y[st * P:(st + 1) * P, :], in_=stt["ysb"])
                stt["st"] = None
            else:
                stt["db"] = db + 1
                stt["g2"] = 0
        else:
            stt["g2"] = g2 + 1

    prev = None
    for qb, g in blocks:
        qsl = slice(qb * NB, (qb + 1) * NB)
        pt = ptpool.tile([P, nt, NB], bf16, tag="pt")
        ps_o = psum.tile([P, NB], f32, tag="o", bufs=2)
        racc = rpool.tile([P, NB], fp16, tag="racc")
        for kj in range(nkp):
            ps_s = psum.tile([P, 2, NB], f32, tag="s", bufs=2)
            for i in range(2):
                ki = 2 * kj + i
                nc.tensor.matmul(
                    ps_s[:, i, :], lhsT=kt_sb[:, ki * P:(ki + 1) * P],
                    rhs=qt_sb[:, g, qsl], start=True, stop=True,
                )
            nc.scalar.activation(
                out=pt[:, 2 * kj:2 * kj + 2, :], in_=ps_s, func=Exp)
            # rowsum of this block's fresh exp pair on DVE (fp16 cascade)
            if kj == 0:
                nc.vector.tensor_add(racc, pt[:, 0, :], pt[:, 1, :])
            else:
                nc.vector.tensor_add(racc, racc, pt[:, 2 * kj, :])
                nc.vector.tensor_add(racc, racc, pt[:, 2 * kj + 1, :])
            if prev is not None:
                ppt = prev[2]
                pps_o = prev[3]
                for i in range(2):
                    ki = 2 * kj + i
                    nc.tensor.matmul(
                        pps_o, lhsT=v_sb[:, ki, :], rhs=ppt[:, ki, :],
                        start=(ki == 0), stop=(ki == nt - 1),
                    )
            op_step()
            op_step()
        if prev is not None:
            finish_block(prev)
            if prev[1] == GRP - 1:
                pqb = prev[0]
                sts_pending.extend(range(pqb * (NB // P), (pqb + 1) * (NB // P)))
        prev = (qb, g, pt, ps_o, racc)
    # drain last block
    qb, g, pt, ps_o, racc = prev
    for ki in range(nt):
        nc.tensor.matmul(
            ps_o, lhsT=v_sb[:, ki, :], rhs=pt[:, ki, :],
            start=(ki == 0), stop=(ki == nt - 1),
        )
        op_step()
    finish_block(prev)
    sts_pending.extend(range(qb * (NB // P), (qb + 1) * (NB // P)))
    while sts_pending or op_state["st"] is not None:
        op_step()


def build_program(s=S, d=D):
    import concourse.tile as tile
    from concourse import bacc, mybir

    nc = bacc.Bacc("TRN2", target_bir_lowering=False, debug=False)
    bf16 = mybir.dt.bfloat16
    fp8 = mybir.dt.float8e4
    f32 = mybir.dt.float32
    aps = {}
    for nm in ["xth", "xtl"]:
        aps[nm] = nc.dram_tensor(nm, [d, s], fp8, kind="ExternalInput").ap()
    for nm in ["wqh", "wql"]:
        aps[nm] = nc.dram_tensor(nm, [d, EH], fp8, kind="ExternalInput").ap()
    for nm in ["wkh", "wkl", "wvh", "wvl"]:
        aps[nm] = nc.dram_tensor(nm, [d, DH], fp8, kind="ExternalInput").ap()
    aps["wo"] = nc.dram_tensor("wo", [EH, d], bf16, kind="ExternalInput").ap()
    aps["bq"] = nc.dram_tensor("bq", [EH], f32, kind="ExternalInput").ap()
    aps["bk"] = nc.dram_tensor("bk", [DH], f32, kind="ExternalInput").ap()
    aps["bv"] = nc.dram_tensor("bv", [DH], f32, kind="ExternalInput").ap()
    aps["y"] = nc.dram_tensor("y", [s, d], bf16, kind="ExternalOutput").ap()
    with tile.TileContext(nc) as tc:
        with ExitStack() as ctx:
            _emit(ctx, tc, aps, s=s, d=d)
    nc.compile()
    return nc


def _two_level(a, sc):
    f8 = ml_dtypes.float8_e4m3
    hi = (a * sc).astype(f8)
    lo = ((a * sc) - hi.astype(np.float32)).astype(f8)
    return hi, lo


def make_in_maps(x, Wq, bq, Wk, bk, Wv, bv, Wo, bo):
    bf = ml_dtypes.bfloat16
    in_maps = []
    xparts = []
    for b in range(B):
        xparts.append(_two_level(np.ascontiguousarray(x[b].T), XSC))
    for b in range(B):
        xth_b, xtl_b = xparts[b]
        for h in range(KV):
            wqh, wql = _two_level(Wq[:, h * EH:(h + 1) * EH], WSC)
            wkh, wkl = _two_level(Wk[:, h * DH:(h + 1) * DH], WSC)
            wvh, wvl = _two_level(Wv[:, h * DH:(h + 1) * DH], WSC)
            in_maps.append({
                "xth": xth_b, "xtl": xtl_b,
                "wqh": wqh, "wql": wql,
                "wkh": wkh, "wkl": wkl,
                "wvh": wvh, "wvl": wvl,
                "wo": np.ascontiguousarray(Wo[h * EH:(h + 1) * EH, :]).astype(bf),
                "bq": (bq[h * EH:(h + 1) * EH] * SCALE).astype(np.float32),
                "bk": np.ascontiguousarray(bk[h * DH:(h + 1) * DH]).astype(np.float32),
                "bv": np.ascontiguousarray(bv[h * DH:(h + 1) * DH]).astype(np.float32),
            })
    return in_maps


_PROG = None


def _get_program():
    global _PROG
    if _PROG is None:
        _PROG = build_program()
    return _PROG


def run_cores(in_maps, trace=False, **kw):
    from concourse.bass_utils import run_bass_kernel_spmd
    nc = _get_program()
    return run_bass_kernel_spmd(nc, in_maps, list(range(8)), trace=trace, **kw)


def kernel(**inputs):
    x = np.asarray(inputs["x"], dtype=np.float32)
    Wq = np.asarray(inputs["Wq"], dtype=np.float32)
    bq = np.asarray(inputs["bq"], dtype=np.float32)
    Wk = np.asarray(inputs["Wk"], dtype=np.float32)
    bk = np.asarray(inputs["bk"], dtype=np.float32)
    Wv = np.asarray(inputs["Wv"], dtype=np.float32)
    bv = np.asarray(inputs["bv"], dtype=np.float32)
    Wo = np.asarray(inputs["Wo"], dtype=np.float32)
    bo = np.asarray(inputs["bo"], dtype=np.float32)

    in_maps = make_in_maps(x, Wq, bq, Wk, bk, Wv, bv, Wo, bo)
    res = run_cores(in_maps)
    out = np.empty((B, S, D), dtype=np.float32)
    for b in range(B):
        acc = res.results[b * KV]["y"].astype(np.float32)
        for h in range(1, KV):
            acc = acc + res.results[b * KV + h]["y"].astype(np.float32)
        out[b] = acc + bo[None, :]
    return out
